# revision 15
# baseline (speedup 1.0000x reference)
"""Gated GQA attention block (B=2,S=2048,E=2048,H=16,HKV=2,D=256,RD=64) on 8 TRN2 cores.

Sharding: data-parallel on batch (2 groups of 4 cores); within a group,
tensor-parallel on query heads (4 heads/core). Each core computes its KV head's
k/v projection locally (duplicated across the 2 cores sharing a KV head).
o_proj is row-parallel; the all-reduce over the 4 cores of a group happens on
the host after gather.

All intermediates (q, g, k, v, gated attention output) stay SBUF-resident in
bf16 — no DRAM round trips between projections, attention, and o_proj.  bf16
matmuls run 1 cycle/row at any moving width, which lets the causal diagonal
restrict score/av matmuls to the live query range.  The softmax denominator is
accumulated chunk-wise on the vector engine and reduced with a single
ones-matmul per (head, column) instead of one per key chunk.
"""

import sys

if "/opt/trn_rl_repo" not in sys.path:
    sys.path.insert(0, "/opt/trn_rl_repo")

import ml_dtypes
import numpy as np

import concourse.bass as bass
import concourse.tile as tile
from concourse import bacc, mybir
from concourse.bass_utils import run_bass_kernel_spmd

F32 = mybir.dt.float32
F32R = mybir.dt.float32r
BF16 = mybir.dt.bfloat16
AF = mybir.ActivationFunctionType

S = 2048          # tokens per batch element
E = 2048          # model dim
D = 256           # head dim
RD = 64           # rope dims
NHC = 4           # q heads per core
HD = NHC * D      # per-core head dims (1024)
ECH = E // 128    # 16 contraction chunks
QCH = HD // 128   # 8 per-core q/g/o d-chunks
TT = 4            # 512-wide token tiles
NKC = S // 128    # 16 k chunks
NQC = S // 128    # 16 q chunks (oproj)


def _r(ap):
    return ap if ap.dtype in (F32R, BF16) else ap.bitcast(F32R)


def _body(tc, d):
    nc = tc.nc
    ts = bass.ts
    ds = bass.ds

    from contextlib import ExitStack

    stack = ExitStack()

    p_res = stack.enter_context(tc.tile_pool(name="res", bufs=1))
    kt = p_res.tile([128, 2, S], BF16, tag="kt")
    vt = p_res.tile([128, NKC, D], BF16, tag="vt")
    qT = p_res.tile([128, QCH, S], BF16, tag="qT")
    gT = p_res.tile([128, QCH, S], BF16, tag="gT")

    p_c = stack.enter_context(tc.tile_pool(name="const", bufs=1))
    ones = p_c.tile([128, 128], F32R, tag="ones")
    nc.scalar.dma_start(ones[:], d["ones"].ap())
    tri = p_c.tile([128, 128], BF16, tag="tri")
    nc.scalar.dma_start(tri[:], d["tri"].ap())
    rotm = p_c.tile([RD, RD], BF16, tag="rotm")
    nc.scalar.dma_start(rotm[:], d["rotm"].ap())
    cos_t = p_c.tile([RD, S], F32, tag="cos")
    nc.scalar.dma_start(cos_t[:], d["cost"].ap())
    sin_t = p_c.tile([RD, S], F32, tag="sin")
    nc.scalar.dma_start(sin_t[:], d["sint"].ap())

    # ---------------- Phase 1: projections ----------------
    with (
        tc.tile_pool(name="psum1", bufs=8, space="PSUM") as psum,
        tc.tile_pool(name="xt", bufs=1) as p_xt,
        tc.tile_pool(name="w", bufs=2) as p_w,
        tc.tile_pool(name="wv", bufs=1) as p_wv,
        tc.tile_pool(name="rtmp", bufs=2) as p_rtmp,
    ):
        wv_t = p_wv.tile([128, ECH, D], BF16, tag="wv")
        for wh in range(4):
            nc.gpsimd.dma_start(
                wv_t[:, ts(wh, ECH // 4), :], d["wv"].ap()[:, ts(wh, ECH // 4), :]
            )

        # Split the x load across both HWDGE rings (sync + scalar) to halve
        # the phase-1 DMA ramp.
        xt = p_xt.tile([128, ECH, S], BF16, tag="xt")
        for ec in range(ECH):
            eng = nc.sync if ec % 2 == 0 else nc.scalar
            eng.dma_start(xt[:, ec, :], d["xt"].ap()[:, ec, :])

        def rope(sl, t):
            # sl[0:64, :512] holds rope dims (partition = d), bf16 in SBUF.
            # rot = R @ x via PE, then sl[0:RD] = x*cos + rot*sin.
            rp = psum.tile([RD, 512], F32, tag="ps")
            nc.tensor.matmul(rp[:], rotm[:], sl[0:RD, :], start=True, stop=True)
            tmp = p_rtmp.tile([RD, 512], F32, tag="rt")
            nc.vector.tensor_mul(tmp[:], sl[0:RD, :], cos_t[:, ts(t, 512)])
            nc.vector.tensor_mul(sl[0:RD, :], rp[:], sin_t[:, ts(t, 512)])
            nc.vector.tensor_add(sl[0:RD, :], sl[0:RD, :], tmp[:])

        def proj_chunk(w_ap, dst, idx, kind):
            wt = p_w.tile([128, ECH, 128], BF16, tag="w")
            nc.scalar.dma_start(wt[:], w_ap)
            pss = []
            for t in range(TT):
                pt = psum.tile([128, 512], F32, tag="ps")
                pss.append(pt)
            for ec in range(ECH):
                for t in range(TT):
                    nc.tensor.matmul(
                        pss[t][:],
                        wt[:, ec, :],
                        xt[:, ec, ts(t, 512)],
                        start=(ec == 0),
                        stop=(ec == ECH - 1),
                    )
            for t in range(TT):
                sl = dst[:, idx, ts(t, 512)]
                if kind == "g":
                    nc.scalar.activation(sl, pss[t][:], AF.Sigmoid)
                else:
                    nc.scalar.copy(sl, pss[t][:])
                    if kind in ("k", "q") and idx % 2 == 0:
                        rope(sl, t)

        # v first, ec-outer in two 8-bank PSUM waves: its matmuls consume each
        # xt chunk as it streams in, filling the DMA ramp.
        for wave in range(2):
            pss = []
            for i in range(8):
                pv = psum.tile([128, D], F32, tag="ps")
                pss.append(pv)
            for ec in range(ECH):
                for i in range(8):
                    tcn = wave * 8 + i
                    nc.tensor.matmul(
                        pss[i][:],
                        xt[:, ec, ts(tcn, 128)],
                        wv_t[:, ec, :],
                        start=(ec == 0),
                        stop=(ec == ECH - 1),
                    )
            for i in range(8):
                tcn = wave * 8 + i
                nc.scalar.copy(vt[:, tcn, :], pss[i][:])

        for j in range(2):
            proj_chunk(d["wk"].ap()[j], kt, j, "k")

        for h in range(NHC):
            for jj in (2 * h, 2 * h + 1):
                proj_chunk(d["wq"].ap()[jj], qT, jj, "q")
            for jj in (2 * h, 2 * h + 1):
                proj_chunk(d["wg"].ap()[jj], gT, jj, "g")

    # ---------------- Phase 2: attention + o_proj ----------------
    with (
        tc.tile_pool(name="pav", bufs=2, space="PSUM") as p_av,
        tc.tile_pool(name="pt1", bufs=4, space="PSUM") as p_t1,
        tc.tile_pool(name="wo", bufs=1) as p_wo,
        tc.tile_pool(name="gat", bufs=2) as p_gat,
        tc.tile_pool(name="ex", bufs=8) as p_ex,
        tc.tile_pool(name="acc", bufs=4) as p_acc,
        tc.tile_pool(name="vec", bufs=4) as p_vec,
        tc.tile_pool(name="ob", bufs=4) as p_ob,
    ):
        wo_t = p_wo.tile([128, QCH, E], BF16, tag="wo")
        nc.gpsimd.dma_start(wo_t[:], d["wo"].ap())

        def oproj_qc(qq, gatc, q4):
            # One 128-token block of o_proj; et pairs so each gat stationary
            # is loaded once per pair while PSUM footprint stays at 2 banks.
            qc = 4 * qq + q4
            for eh in range(2):
                ops = []
                for e2 in range(2):
                    op = p_t1.tile([128, 512], F32, tag="t1")
                    ops.append(op)
                for hc in range(QCH):
                    for e2 in range(2):
                        nc.tensor.matmul(
                            ops[e2][:],
                            gatc[:, hc, ts(q4, 128)],
                            wo_t[:, hc, ts(2 * eh + e2, 512)],
                            start=(hc == 0),
                            stop=(hc == QCH - 1),
                        )
                for e2 in range(2):
                    et = 2 * eh + e2
                    ob = p_ob.tile([128, 512], F32, tag="ob")
                    nc.scalar.copy(ob[:], ops[e2][:])
                    eng = nc.sync if et % 2 == 0 else nc.scalar
                    eng.dma_start(d["out"].ap()[qc][:, ts(et, 512)], ob[:])

        def attn_head(qq, h, gatc):
            nk = 4 * qq + 4
            av2 = p_av.tile([128, 2, 512], F32, tag="av2")
            acc_a = p_acc.tile([128, 512], F32R, tag="acc")
            # Column 0's odd chunks are all partial-width (diagonal), so a
            # full-width copy-init of a second partial chain would read
            # stale data there; use a single chain for that short column.
            split = qq > 0
            if split:
                acc_b = p_acc.tile([128, 512], F32R, tag="acc")
            else:
                acc_b = None
            na = nb = 0
            for kk in range(nk):
                dg = kk - 4 * qq
                lo = 128 * dg if dg > 0 else 0
                w = 512 - lo
                sp = p_t1.tile([128, 512], F32, tag="t1")
                nc.tensor.matmul(
                    sp[:, ds(lo, w)],
                    kt[:, 0, ts(kk, 128)],
                    qT[:, 2 * h, ds(qq * 512 + lo, w)],
                    start=True, stop=False,
                )
                nc.tensor.matmul(
                    sp[:, ds(lo, w)],
                    kt[:, 1, ts(kk, 128)],
                    qT[:, 2 * h + 1, ds(qq * 512 + lo, w)],
                    start=False, stop=True,
                )
                ex = p_ex.tile([128, 512], BF16, tag="ex")
                nc.scalar.activation(
                    ex[:, ds(lo, w)], sp[:, ds(lo, w)], AF.Exp, scale=0.0625
                )
                if dg >= 0:
                    nc.vector.tensor_mul(
                        ex[:, ds(lo, 128)], ex[:, ds(lo, 128)], tri[:]
                    )
                # Two interleaved softmax-denominator partials: even chunks
                # accumulate on DVE, odd chunks on GpSimd — halves the serial
                # chain and keeps it off the PE-feeding DVE queue.
                if kk % 2 == 0 or not split:
                    if na == 0:
                        nc.vector.tensor_copy(acc_a[:], ex[:])
                    else:
                        nc.vector.tensor_add(
                            acc_a[:, ds(lo, w)], acc_a[:, ds(lo, w)],
                            ex[:, ds(lo, w)],
                        )
                    na += 1
                else:
                    if nb == 0:
                        nc.vector.tensor_copy(acc_b[:], ex[:])
                    else:
                        nc.vector.tensor_add(
                            acc_b[:, ds(lo, w)], acc_b[:, ds(lo, w)],
                            ex[:, ds(lo, w)],
                        )
                    nb += 1
                st, en = (kk == 0), (kk == nk - 1)
                nc.tensor.matmul(
                    av2[:, 0, ds(lo, w)], vt[:, kk, 0:128], ex[:, ds(lo, w)],
                    start=st, stop=en,
                )
                nc.tensor.matmul(
                    av2[:, 1, ds(lo, w)], vt[:, kk, 128:256], ex[:, ds(lo, w)],
                    start=st, stop=en,
                )
            sm = p_t1.tile([128, 512], F32, tag="t1")
            nc.tensor.matmul(sm[:], ones[:], acc_a[:], start=True, stop=not split)
            if split:
                nc.tensor.matmul(sm[:], ones[:], acc_b[:], start=False, stop=True)
            rec = p_vec.tile([128, 1, 512], F32, tag="rec")
            nc.vector.reciprocal(rec[:, 0, :], sm[:])
            g1 = p_vec.tile([128, 2, 512], F32, tag="g1")
            nc.vector.tensor_mul(
                g1[:], av2[:], gT[:, ds(2 * h, 2), ts(qq, 512)]
            )
            g1b, recb = bass.broadcast_tensor_aps(g1[:], rec[:])
            nc.vector.tensor_mul(gatc[:, ds(2 * h, 2), :], g1b, recb)

        prev = None
        for qq in (3, 2, 1, 0):
            gatc = p_gat.tile([128, QCH, 512], BF16, tag="gat")
            for h in range(NHC):
                attn_head(qq, h, gatc)
                # o_proj of the previous column interleaves one 128-token
                # block per head: independent PE work that fills the
                # exp/mask dependency bubbles of the attention chain.
                if prev is not None:
                    oproj_qc(prev[0], prev[1], h)
            prev = (qq, gatc)
        for q4 in range(4):
            oproj_qc(prev[0], prev[1], q4)

    stack.close()


def build_nc():
    nc = bacc.Bacc("TRN2", target_bir_lowering=False, debug=False)
    d = {}
    d["xt"] = nc.dram_tensor("xt", [128, ECH, S], BF16, kind="ExternalInput")
    d["wq"] = nc.dram_tensor("wq", [QCH, 128, ECH, 128], BF16, kind="ExternalInput")
    d["wg"] = nc.dram_tensor("wg", [QCH, 128, ECH, 128], BF16, kind="ExternalInput")
    d["wk"] = nc.dram_tensor("wk", [2, 128, ECH, 128], BF16, kind="ExternalInput")
    d["wv"] = nc.dram_tensor("wv", [128, ECH, D], BF16, kind="ExternalInput")
    d["wo"] = nc.dram_tensor("wo", [128, QCH, E], BF16, kind="ExternalInput")
    d["cost"] = nc.dram_tensor("cost", [RD, S], F32, kind="ExternalInput")
    d["sint"] = nc.dram_tensor("sint", [RD, S], F32, kind="ExternalInput")
    d["tri"] = nc.dram_tensor("tri", [128, 128], BF16, kind="ExternalInput")
    d["rotm"] = nc.dram_tensor("rotm", [RD, RD], BF16, kind="ExternalInput")
    d["ones"] = nc.dram_tensor("ones", [128, 128], F32R, kind="ExternalInput")
    d["out"] = nc.dram_tensor("out", [NQC, 128, E], F32, kind="ExternalOutput")
    with tile.TileContext(nc) as tc:
        _body(tc, d)
    nc.compile()
    return nc


_NC_CACHE = None


def _get_nc():
    global _NC_CACHE
    if _NC_CACHE is None:
        _NC_CACHE = build_nc()
    return _NC_CACHE


def _rope_tables():
    inv = 1.0 / (10000.0 ** (np.arange(0, RD, 2, dtype=np.float32) / np.float32(RD)))
    t = np.arange(S, dtype=np.float32)
    freqs = np.outer(t, inv).astype(np.float32)          # [S, RD/2]
    emb = np.concatenate([freqs, freqs], axis=1)         # [S, RD]
    return (
        np.ascontiguousarray(np.cos(emb).astype(np.float32).T),
        np.ascontiguousarray(np.sin(emb).astype(np.float32).T),
    )


def _rotm():
    r = np.zeros((RD, RD), dtype=np.float32)  # r[j, d] = R[d, j], rot = R @ x
    half = RD // 2
    for dd in range(half):
        r[dd + half, dd] = -1.0
    for dd in range(half, RD):
        r[dd - half, dd] = 1.0
    return r


def _tri():
    p = np.arange(128)[:, None]
    j = np.arange(128)[None, :]
    return (p <= j).astype(ml_dtypes.bfloat16)


def _prep_in_maps(hidden_states, Wq, Wk, Wv, Wg, Wo):
    cosT, sinT = _rope_tables()
    tri = _tri()
    maps = []
    for c in range(8):
        b, t = c // 4, c % 4
        hq0, kvh = 4 * t, (t // 2)
        cols = slice(hq0 * D, (hq0 + NHC) * D)
        kcols = slice(kvh * D, (kvh + 1) * D)
        x = hidden_states[b]  # [S, E]
        m = {
            "xt": np.ascontiguousarray(
                x.T.reshape(ECH, 128, S).transpose(1, 0, 2)
            ).astype(ml_dtypes.bfloat16),
            "wq": np.ascontiguousarray(
                Wq[:, cols].reshape(ECH, 128, QCH, 128).transpose(2, 1, 0, 3)
            ).astype(ml_dtypes.bfloat16),
            "wg": np.ascontiguousarray(
                Wg[:, cols].reshape(ECH, 128, QCH, 128).transpose(2, 1, 0, 3)
            ).astype(ml_dtypes.bfloat16),
            "wk": np.ascontiguousarray(
                Wk[:, kcols].reshape(ECH, 128, 2, 128).transpose(2, 1, 0, 3)
            ).astype(ml_dtypes.bfloat16),
            "wv": np.ascontiguousarray(
                Wv[:, kcols].reshape(ECH, 128, D).transpose(1, 0, 2)
            ).astype(ml_dtypes.bfloat16),
            "wo": np.ascontiguousarray(
                Wo[cols, :].reshape(QCH, 128, E).transpose(1, 0, 2)
            ).astype(ml_dtypes.bfloat16),
            "cost": cosT,
            "sint": sinT,
            "tri": tri,
            "rotm": _rotm().astype(ml_dtypes.bfloat16),
            "ones": np.ones((128, 128), dtype=np.float32),
        }
        maps.append(m)
    return maps


def _run(inputs, trace=False, trace_cores=None, tmpdir=None):
    nc = _get_nc()
    in_maps = _prep_in_maps(**inputs)
    kw = {}
    if trace:
        kw = dict(trace=True, trace_cores=trace_cores, tmpdir=tmpdir)
    res = run_bass_kernel_spmd(nc, in_maps, list(range(8)), **kw)
    outs = [res.results[c]["out"].reshape(S, E) for c in range(8)]
    full = np.stack(
        [
            outs[0] + outs[1] + outs[2] + outs[3],
            outs[4] + outs[5] + outs[6] + outs[7],
        ]
    ).astype(np.float32)
    return full, res


def kernel(hidden_states, Wq, Wk, Wv, Wg, Wo):
    full, _ = _run(
        dict(hidden_states=np.asarray(hidden_states, dtype=np.float32),
             Wq=np.asarray(Wq, dtype=np.float32),
             Wk=np.asarray(Wk, dtype=np.float32),
             Wv=np.asarray(Wv, dtype=np.float32),
             Wg=np.asarray(Wg, dtype=np.float32),
             Wo=np.asarray(Wo, dtype=np.float32))
    )
    return full


if __name__ == "__main__":
    build_nc()
    print("build OK")


# revision 22
# speedup vs baseline: 1.0870x; 1.0870x over previous
"""Gated GQA attention block (B=2,S=2048,E=2048,H=16,HKV=2,D=256,RD=64) on 8 TRN2 cores.

Sharding: data-parallel on batch (2 groups of 4 cores); within a group,
tensor-parallel on query heads (4 heads/core). Each core computes its KV head's
k/v projection locally (duplicated across the 2 cores sharing a KV head).
o_proj is row-parallel; the all-reduce over the 4 cores of a group happens on
the host after gather.

All intermediates (q, g, k, v, gated attention output) stay SBUF-resident in
bf16 — no DRAM round trips between projections, attention, and o_proj.  bf16
matmuls run 1 cycle/row at any moving width, which lets the causal diagonal
restrict score/av matmuls to the live query range.  The softmax denominator is
accumulated chunk-wise on the vector engine and reduced with a single
ones-matmul per (head, column) instead of one per key chunk.
"""

import sys

if "/opt/trn_rl_repo" not in sys.path:
    sys.path.insert(0, "/opt/trn_rl_repo")

import ml_dtypes
import numpy as np

import concourse.bass as bass
import concourse.tile as tile
from concourse import bacc, mybir
from concourse.bass_utils import run_bass_kernel_spmd

F32 = mybir.dt.float32
F32R = mybir.dt.float32r
BF16 = mybir.dt.bfloat16
AF = mybir.ActivationFunctionType

S = 2048          # tokens per batch element
E = 2048          # model dim
D = 256           # head dim
RD = 64           # rope dims
NHC = 4           # q heads per core
HD = NHC * D      # per-core head dims (1024)
ECH = E // 128    # 16 contraction chunks
QCH = HD // 128   # 8 per-core q/g/o d-chunks
TT = 4            # 512-wide token tiles
NKC = S // 128    # 16 k chunks
NQC = S // 128    # 16 q chunks (oproj)


def _r(ap):
    return ap if ap.dtype in (F32R, BF16) else ap.bitcast(F32R)


def _body(tc, d):
    nc = tc.nc
    ts = bass.ts
    ds = bass.ds

    from contextlib import ExitStack

    stack = ExitStack()

    p_res = stack.enter_context(tc.tile_pool(name="res", bufs=1))
    kt = p_res.tile([128, 2, S], BF16, tag="kt")
    vt = p_res.tile([128, NKC, D], BF16, tag="vt")
    qT = p_res.tile([128, QCH, S], BF16, tag="qT")
    gT = p_res.tile([128, QCH, S], BF16, tag="gT")

    p_c = stack.enter_context(tc.tile_pool(name="const", bufs=1))
    ones = p_c.tile([128, 128], F32R, tag="ones")
    nc.scalar.dma_start(ones[:], d["ones"].ap())
    tri = p_c.tile([128, 128], BF16, tag="tri")
    nc.scalar.dma_start(tri[:], d["tri"].ap())
    cos_t = p_c.tile([RD, S], F32, tag="cos")
    nc.scalar.dma_start(cos_t[:], d["cost"].ap())
    sin_t = p_c.tile([RD, S], F32, tag="sin")
    nc.scalar.dma_start(sin_t[:], d["sint"].ap())

    # ---------------- Phase 1: projections ----------------
    with (
        tc.tile_pool(name="psum1", bufs=8, space="PSUM") as psum,
        tc.tile_pool(name="xt", bufs=1) as p_xt,
        tc.tile_pool(name="w", bufs=2) as p_w,
        tc.tile_pool(name="wv", bufs=1) as p_wv,
        tc.tile_pool(name="rtmp", bufs=2) as p_rtmp,
    ):
        wv_t = p_wv.tile([128, ECH, D], BF16, tag="wv")
        for wh in range(4):
            nc.gpsimd.dma_start(
                wv_t[:, ts(wh, ECH // 4), :], d["wv"].ap()[:, ts(wh, ECH // 4), :]
            )

        # Split the x load across both HWDGE rings (sync + scalar) to halve
        # the phase-1 DMA ramp.
        xt = p_xt.tile([128, ECH, S], BF16, tag="xt")
        for ec in range(ECH):
            eng = nc.sync if ec % 2 == 0 else nc.scalar
            eng.dma_start(xt[:, ec, :], d["xt"].ap()[:, ec, :])

        def rope(sl, t):
            # sl[0:64, :512] holds rope dims (partition = d), bf16 in SBUF.
            # The half-rotation is a partition swap done by SBUF->SBUF DMA
            # (sign is folded into the sin table), keeping rope off the PE.
            half = RD // 2
            rot = p_rtmp.tile([RD, 512], BF16, tag="rot")
            nc.sync.dma_start(rot[0:half, :], sl[half:RD, :])
            nc.sync.dma_start(rot[half:RD, :], sl[0:half, :])
            tmp = p_rtmp.tile([RD, 512], F32, tag="rt")
            nc.vector.tensor_mul(tmp[:], sl[0:RD, :], cos_t[:, ts(t, 512)])
            nc.vector.tensor_mul(sl[0:RD, :], rot[:], sin_t[:, ts(t, 512)])
            nc.vector.tensor_add(sl[0:RD, :], sl[0:RD, :], tmp[:])

        def proj_chunk(w_ap, dst, idx, kind):
            wt = p_w.tile([128, ECH, 128], BF16, tag="w")
            nc.scalar.dma_start(wt[:], w_ap)
            pss = []
            for t in range(TT):
                pt = psum.tile([128, 512], F32, tag="ps")
                pss.append(pt)
            for ec in range(ECH):
                for t in range(TT):
                    nc.tensor.matmul(
                        pss[t][:],
                        wt[:, ec, :],
                        xt[:, ec, ts(t, 512)],
                        start=(ec == 0),
                        stop=(ec == ECH - 1),
                    )
            for t in range(TT):
                sl = dst[:, idx, ts(t, 512)]
                if kind == "g":
                    nc.scalar.activation(sl, pss[t][:], AF.Sigmoid)
                else:
                    nc.scalar.copy(sl, pss[t][:])
                    if kind in ("k", "q") and idx % 2 == 0:
                        rope(sl, t)

        # v first, ec-outer in two 8-bank PSUM waves: its matmuls consume each
        # xt chunk as it streams in, filling the DMA ramp.
        for wave in range(2):
            pss = []
            for i in range(8):
                pv = psum.tile([128, D], F32, tag="ps")
                pss.append(pv)
            for ec in range(ECH):
                for i in range(8):
                    tcn = wave * 8 + i
                    nc.tensor.matmul(
                        pss[i][:],
                        xt[:, ec, ts(tcn, 128)],
                        wv_t[:, ec, :],
                        start=(ec == 0),
                        stop=(ec == ECH - 1),
                    )
            for i in range(8):
                tcn = wave * 8 + i
                nc.scalar.copy(vt[:, tcn, :], pss[i][:])

        for j in range(2):
            proj_chunk(d["wk"].ap()[j], kt, j, "k")

        for h in range(NHC):
            for jj in (2 * h, 2 * h + 1):
                proj_chunk(d["wq"].ap()[jj], qT, jj, "q")
            for jj in (2 * h, 2 * h + 1):
                proj_chunk(d["wg"].ap()[jj], gT, jj, "g")

    # ---------------- Phase 2: attention + o_proj ----------------
    with (
        tc.tile_pool(name="pav", bufs=2, space="PSUM") as p_av,
        tc.tile_pool(name="pt1", bufs=4, space="PSUM") as p_t1,
        tc.tile_pool(name="wo", bufs=1) as p_wo,
        tc.tile_pool(name="gat", bufs=2) as p_gat,
        tc.tile_pool(name="ex", bufs=8) as p_ex,
        tc.tile_pool(name="acc", bufs=4) as p_acc,
        tc.tile_pool(name="vec", bufs=4) as p_vec,
        tc.tile_pool(name="ob", bufs=4) as p_ob,
    ):
        wo_t = p_wo.tile([128, QCH, E], BF16, tag="wo")
        nc.gpsimd.dma_start(wo_t[:], d["wo"].ap())

        def oproj_qc(qq, gatc, q4):
            # One 128-token block of o_proj; et pairs so each gat stationary
            # is loaded once per pair while PSUM footprint stays at 2 banks.
            qc = 4 * qq + q4
            for eh in range(2):
                ops = []
                for e2 in range(2):
                    op = p_t1.tile([128, 512], F32, tag="t1")
                    ops.append(op)
                for hc in range(QCH):
                    for e2 in range(2):
                        nc.tensor.matmul(
                            ops[e2][:],
                            gatc[:, hc, ts(q4, 128)],
                            wo_t[:, hc, ts(2 * eh + e2, 512)],
                            start=(hc == 0),
                            stop=(hc == QCH - 1),
                        )
                for e2 in range(2):
                    et = 2 * eh + e2
                    ob = p_ob.tile([128, 512], F32, tag="ob")
                    nc.scalar.copy(ob[:], ops[e2][:])
                    eng = nc.sync if et % 2 == 0 else nc.scalar
                    eng.dma_start(d["out"].ap()[qc][:, ts(et, 512)], ob[:])

        def attn_head(qq, h, gatc):
            nk = 4 * qq + 4
            av2 = p_av.tile([128, 2, 512], F32, tag="av2")
            acc_a = p_acc.tile([128, 512], F32R, tag="acc")
            # Column 0's odd chunks are all partial-width (diagonal), so a
            # full-width copy-init of a second partial chain would read
            # stale data there; use a single chain for that short column.
            split = qq > 0
            if split:
                acc_b = p_acc.tile([128, 512], F32R, tag="acc")
            else:
                acc_b = None
            na = nb = 0
            for kk in range(nk):
                dg = kk - 4 * qq
                lo = 128 * dg if dg > 0 else 0
                w = 512 - lo
                sp = p_t1.tile([128, 512], F32, tag="t1")
                nc.tensor.matmul(
                    sp[:, ds(lo, w)],
                    kt[:, 0, ts(kk, 128)],
                    qT[:, 2 * h, ds(qq * 512 + lo, w)],
                    start=True, stop=False,
                )
                nc.tensor.matmul(
                    sp[:, ds(lo, w)],
                    kt[:, 1, ts(kk, 128)],
                    qT[:, 2 * h + 1, ds(qq * 512 + lo, w)],
                    start=False, stop=True,
                )
                ex = p_ex.tile([128, 512], BF16, tag="ex")
                nc.scalar.activation(
                    ex[:, ds(lo, w)], sp[:, ds(lo, w)], AF.Exp, scale=0.0625
                )
                if dg >= 0:
                    nc.vector.tensor_mul(
                        ex[:, ds(lo, 128)], ex[:, ds(lo, 128)], tri[:]
                    )
                # Two interleaved softmax-denominator partials: even chunks
                # accumulate on DVE, odd chunks on GpSimd — halves the serial
                # chain and keeps it off the PE-feeding DVE queue.
                # Denominator partials accumulate on GpSimd: keeps the adds
                # out of the DVE FIFO so the tri mask (which gates av) is
                # never queued behind them.
                if kk % 2 == 0 or not split:
                    if na == 0:
                        nc.gpsimd.tensor_copy(acc_a[:], ex[:])
                    else:
                        nc.gpsimd.tensor_add(
                            acc_a[:, ds(lo, w)], acc_a[:, ds(lo, w)],
                            ex[:, ds(lo, w)],
                        )
                    na += 1
                else:
                    if nb == 0:
                        nc.gpsimd.tensor_copy(acc_b[:], ex[:])
                    else:
                        nc.gpsimd.tensor_add(
                            acc_b[:, ds(lo, w)], acc_b[:, ds(lo, w)],
                            ex[:, ds(lo, w)],
                        )
                    nb += 1
                st, en = (kk == 0), (kk == nk - 1)
                nc.tensor.matmul(
                    av2[:, 0, ds(lo, w)], vt[:, kk, 0:128], ex[:, ds(lo, w)],
                    start=st, stop=en,
                )
                nc.tensor.matmul(
                    av2[:, 1, ds(lo, w)], vt[:, kk, 128:256], ex[:, ds(lo, w)],
                    start=st, stop=en,
                )
            sm = p_t1.tile([128, 512], F32, tag="t1")
            nc.tensor.matmul(sm[:], ones[:], acc_a[:], start=True, stop=not split)
            if split:
                nc.tensor.matmul(sm[:], ones[:], acc_b[:], start=False, stop=True)
            rec = p_vec.tile([128, 1, 512], F32, tag="rec")
            nc.vector.reciprocal(rec[:, 0, :], sm[:])
            g1 = p_vec.tile([128, 2, 512], F32, tag="g1")
            nc.vector.tensor_mul(
                g1[:], av2[:], gT[:, ds(2 * h, 2), ts(qq, 512)]
            )
            g1b, recb = bass.broadcast_tensor_aps(g1[:], rec[:])
            nc.vector.tensor_mul(gatc[:, ds(2 * h, 2), :], g1b, recb)

        prev = None
        for qq in (3, 2, 1, 0):
            gatc = p_gat.tile([128, QCH, 512], BF16, tag="gat")
            for h in range(NHC):
                attn_head(qq, h, gatc)
                # o_proj of the previous column interleaves one 128-token
                # block per head: independent PE work that fills the
                # exp/mask dependency bubbles of the attention chain.
                if prev is not None:
                    oproj_qc(prev[0], prev[1], h)
            prev = (qq, gatc)
        for q4 in range(4):
            oproj_qc(prev[0], prev[1], q4)

    stack.close()


def build_nc():
    nc = bacc.Bacc("TRN2", target_bir_lowering=False, debug=False)
    d = {}
    d["xt"] = nc.dram_tensor("xt", [128, ECH, S], BF16, kind="ExternalInput")
    d["wq"] = nc.dram_tensor("wq", [QCH, 128, ECH, 128], BF16, kind="ExternalInput")
    d["wg"] = nc.dram_tensor("wg", [QCH, 128, ECH, 128], BF16, kind="ExternalInput")
    d["wk"] = nc.dram_tensor("wk", [2, 128, ECH, 128], BF16, kind="ExternalInput")
    d["wv"] = nc.dram_tensor("wv", [128, ECH, D], BF16, kind="ExternalInput")
    d["wo"] = nc.dram_tensor("wo", [128, QCH, E], BF16, kind="ExternalInput")
    d["cost"] = nc.dram_tensor("cost", [RD, S], F32, kind="ExternalInput")
    d["sint"] = nc.dram_tensor("sint", [RD, S], F32, kind="ExternalInput")
    d["tri"] = nc.dram_tensor("tri", [128, 128], BF16, kind="ExternalInput")
    d["ones"] = nc.dram_tensor("ones", [128, 128], F32R, kind="ExternalInput")
    d["out"] = nc.dram_tensor("out", [NQC, 128, E], F32, kind="ExternalOutput")
    with tile.TileContext(nc) as tc:
        _body(tc, d)
    nc.compile()
    return nc


_NC_CACHE = None


def _get_nc():
    global _NC_CACHE
    if _NC_CACHE is None:
        _NC_CACHE = build_nc()
    return _NC_CACHE


def _rope_tables():
    inv = 1.0 / (10000.0 ** (np.arange(0, RD, 2, dtype=np.float32) / np.float32(RD)))
    t = np.arange(S, dtype=np.float32)
    freqs = np.outer(t, inv).astype(np.float32)          # [S, RD/2]
    emb = np.concatenate([freqs, freqs], axis=1)         # [S, RD]
    sinT = np.ascontiguousarray(np.sin(emb).astype(np.float32).T)
    # The rope rotation's sign lives in the sin table: the kernel builds
    # rot = [x_hi, x_lo] by partition swap and the -1 on the first half
    # of the rotated vector is folded in here.
    sinT[: RD // 2, :] *= -1.0
    return np.ascontiguousarray(np.cos(emb).astype(np.float32).T), sinT


def _tri():
    p = np.arange(128)[:, None]
    j = np.arange(128)[None, :]
    return (p <= j).astype(ml_dtypes.bfloat16)


def _prep_in_maps(hidden_states, Wq, Wk, Wv, Wg, Wo):
    cosT, sinT = _rope_tables()
    tri = _tri()
    maps = []
    for c in range(8):
        b, t = c // 4, c % 4
        hq0, kvh = 4 * t, (t // 2)
        cols = slice(hq0 * D, (hq0 + NHC) * D)
        kcols = slice(kvh * D, (kvh + 1) * D)
        x = hidden_states[b]  # [S, E]
        m = {
            "xt": np.ascontiguousarray(
                x.T.reshape(ECH, 128, S).transpose(1, 0, 2)
            ).astype(ml_dtypes.bfloat16),
            "wq": np.ascontiguousarray(
                Wq[:, cols].reshape(ECH, 128, QCH, 128).transpose(2, 1, 0, 3)
            ).astype(ml_dtypes.bfloat16),
            "wg": np.ascontiguousarray(
                Wg[:, cols].reshape(ECH, 128, QCH, 128).transpose(2, 1, 0, 3)
            ).astype(ml_dtypes.bfloat16),
            "wk": np.ascontiguousarray(
                Wk[:, kcols].reshape(ECH, 128, 2, 128).transpose(2, 1, 0, 3)
            ).astype(ml_dtypes.bfloat16),
            "wv": np.ascontiguousarray(
                Wv[:, kcols].reshape(ECH, 128, D).transpose(1, 0, 2)
            ).astype(ml_dtypes.bfloat16),
            "wo": np.ascontiguousarray(
                Wo[cols, :].reshape(QCH, 128, E).transpose(1, 0, 2)
            ).astype(ml_dtypes.bfloat16),
            "cost": cosT,
            "sint": sinT,
            "tri": tri,
            "ones": np.ones((128, 128), dtype=np.float32),
        }
        maps.append(m)
    return maps


def _run(inputs, trace=False, trace_cores=None, tmpdir=None):
    nc = _get_nc()
    in_maps = _prep_in_maps(**inputs)
    kw = {}
    if trace:
        kw = dict(trace=True, trace_cores=trace_cores, tmpdir=tmpdir)
    res = run_bass_kernel_spmd(nc, in_maps, list(range(8)), **kw)
    outs = [res.results[c]["out"].reshape(S, E) for c in range(8)]
    full = np.stack(
        [
            outs[0] + outs[1] + outs[2] + outs[3],
            outs[4] + outs[5] + outs[6] + outs[7],
        ]
    ).astype(np.float32)
    return full, res


def kernel(hidden_states, Wq, Wk, Wv, Wg, Wo):
    full, _ = _run(
        dict(hidden_states=np.asarray(hidden_states, dtype=np.float32),
             Wq=np.asarray(Wq, dtype=np.float32),
             Wk=np.asarray(Wk, dtype=np.float32),
             Wv=np.asarray(Wv, dtype=np.float32),
             Wg=np.asarray(Wg, dtype=np.float32),
             Wo=np.asarray(Wo, dtype=np.float32))
    )
    return full


if __name__ == "__main__":
    build_nc()
    print("build OK")


# revision 25
# speedup vs baseline: 1.1793x; 1.0849x over previous
"""Gated GQA attention block (B=2,S=2048,E=2048,H=16,HKV=2,D=256,RD=64) on 8 TRN2 cores.

Sharding: data-parallel on batch (2 groups of 4 cores); within a group,
tensor-parallel on query heads (4 heads/core). Each core computes its KV head's
k/v projection locally (duplicated across the 2 cores sharing a KV head).
o_proj is row-parallel; the all-reduce over the 4 cores of a group happens on
the host after gather.

All intermediates (q, g, k, v, gated attention output) stay SBUF-resident in
bf16 — no DRAM round trips between projections, attention, and o_proj.  bf16
matmuls run 1 cycle/row at any moving width, which lets the causal diagonal
restrict score/av matmuls to the live query range.  The softmax denominator is
accumulated chunk-wise on the vector engine and reduced with a single
ones-matmul per (head, column) instead of one per key chunk.
"""

import sys

if "/opt/trn_rl_repo" not in sys.path:
    sys.path.insert(0, "/opt/trn_rl_repo")

import ml_dtypes
import numpy as np

import concourse.bass as bass
import concourse.tile as tile
from concourse import bacc, mybir
from concourse.bass_utils import run_bass_kernel_spmd

F32 = mybir.dt.float32
F32R = mybir.dt.float32r
BF16 = mybir.dt.bfloat16
AF = mybir.ActivationFunctionType

S = 2048          # tokens per batch element
E = 2048          # model dim
D = 256           # head dim
RD = 64           # rope dims
NHC = 4           # q heads per core
HD = NHC * D      # per-core head dims (1024)
ECH = E // 128    # 16 contraction chunks
QCH = HD // 128   # 8 per-core q/g/o d-chunks
TT = 4            # 512-wide token tiles
NKC = S // 128    # 16 k chunks
NQC = S // 128    # 16 q chunks (oproj)


def _r(ap):
    return ap if ap.dtype in (F32R, BF16) else ap.bitcast(F32R)


def _body(tc, d):
    nc = tc.nc
    ts = bass.ts
    ds = bass.ds

    from contextlib import ExitStack

    stack = ExitStack()

    p_res = stack.enter_context(tc.tile_pool(name="res", bufs=1))
    kt = p_res.tile([128, 2, S], BF16, tag="kt")
    vt = p_res.tile([128, NKC, D], BF16, tag="vt")
    qT = p_res.tile([128, QCH, S], BF16, tag="qT")
    gT = p_res.tile([128, QCH, S], BF16, tag="gT")

    p_c = stack.enter_context(tc.tile_pool(name="const", bufs=1))
    ones = p_c.tile([128, 128], F32R, tag="ones")
    nc.scalar.dma_start(ones[:], d["ones"].ap())
    tri = p_c.tile([128, 128], BF16, tag="tri")
    nc.scalar.dma_start(tri[:], d["tri"].ap())
    cos_t = p_c.tile([RD, S], F32, tag="cos")
    nc.scalar.dma_start(cos_t[:], d["cost"].ap())
    sin_t = p_c.tile([RD, S], F32, tag="sin")
    nc.scalar.dma_start(sin_t[:], d["sint"].ap())

    # ---------------- Phase 1: projections ----------------
    with (
        tc.tile_pool(name="psum1", bufs=8, space="PSUM") as psum,
        tc.tile_pool(name="xt", bufs=1) as p_xt,
        tc.tile_pool(name="w", bufs=2) as p_w,
        tc.tile_pool(name="wv", bufs=1) as p_wv,
        tc.tile_pool(name="rtmp", bufs=2) as p_rtmp,
    ):
        wv_t = p_wv.tile([128, ECH, D], BF16, tag="wv")
        for wh in range(4):
            nc.gpsimd.dma_start(
                wv_t[:, ts(wh, ECH // 4), :], d["wv"].ap()[:, ts(wh, ECH // 4), :]
            )

        # Split the x load across both HWDGE rings (sync + scalar) to halve
        # the phase-1 DMA ramp.
        xt = p_xt.tile([128, ECH, S], BF16, tag="xt")
        for ec in range(ECH):
            eng = nc.sync if ec % 2 == 0 else nc.scalar
            eng.dma_start(xt[:, ec, :], d["xt"].ap()[:, ec, :])

        def rope(sl, t):
            # sl[0:64, :512] holds rope dims (partition = d), bf16 in SBUF.
            # The half-rotation is a partition swap done by SBUF->SBUF DMA
            # (sign is folded into the sin table), keeping rope off the PE.
            half = RD // 2
            rot = p_rtmp.tile([RD, 512], BF16, tag="rot")
            nc.sync.dma_start(rot[0:half, :], sl[half:RD, :])
            nc.sync.dma_start(rot[half:RD, :], sl[0:half, :])
            tmp = p_rtmp.tile([RD, 512], F32, tag="rt")
            nc.vector.tensor_mul(tmp[:], sl[0:RD, :], cos_t[:, ts(t, 512)])
            nc.vector.tensor_mul(sl[0:RD, :], rot[:], sin_t[:, ts(t, 512)])
            nc.vector.tensor_add(sl[0:RD, :], sl[0:RD, :], tmp[:])

        def proj_chunk(w_ap, dst, idx, kind):
            wt = p_w.tile([128, ECH, 128], BF16, tag="w")
            nc.gpsimd.dma_start(wt[:], w_ap)
            pss = []
            for t in range(TT):
                pt = psum.tile([128, 512], F32, tag="ps")
                pss.append(pt)
            for ec in range(ECH):
                for t in range(TT):
                    nc.tensor.matmul(
                        pss[t][:],
                        wt[:, ec, :],
                        xt[:, ec, ts(t, 512)],
                        start=(ec == 0),
                        stop=(ec == ECH - 1),
                    )
            for t in range(TT):
                sl = dst[:, idx, ts(t, 512)]
                if kind == "g":
                    nc.scalar.activation(sl, pss[t][:], AF.Sigmoid)
                else:
                    nc.scalar.copy(sl, pss[t][:])
                    if kind in ("k", "q") and idx % 2 == 0:
                        rope(sl, t)

        # v first, ec-outer in two 8-bank PSUM waves: its matmuls consume each
        # xt chunk as it streams in, filling the DMA ramp.
        for wave in range(2):
            pss = []
            for i in range(8):
                pv = psum.tile([128, D], F32, tag="ps")
                pss.append(pv)
            for ec in range(ECH):
                for i in range(8):
                    tcn = wave * 8 + i
                    nc.tensor.matmul(
                        pss[i][:],
                        xt[:, ec, ts(tcn, 128)],
                        wv_t[:, ec, :],
                        start=(ec == 0),
                        stop=(ec == ECH - 1),
                    )
            for i in range(8):
                tcn = wave * 8 + i
                nc.scalar.copy(vt[:, tcn, :], pss[i][:])

        for j in range(2):
            proj_chunk(d["wk"].ap()[j], kt, j, "k")

        for h in range(NHC):
            for jj in (2 * h, 2 * h + 1):
                proj_chunk(d["wq"].ap()[jj], qT, jj, "q")
            for jj in (2 * h, 2 * h + 1):
                proj_chunk(d["wg"].ap()[jj], gT, jj, "g")

    # ---------------- Phase 2: attention + o_proj ----------------
    with (
        tc.tile_pool(name="pav", bufs=2, space="PSUM") as p_av,
        tc.tile_pool(name="pt1", bufs=4, space="PSUM") as p_t1,
        tc.tile_pool(name="wo", bufs=1) as p_wo,
        tc.tile_pool(name="gat", bufs=2) as p_gat,
        tc.tile_pool(name="ex", bufs=10) as p_ex,
        tc.tile_pool(name="acc", bufs=4) as p_acc,
        tc.tile_pool(name="vec", bufs=4) as p_vec,
        tc.tile_pool(name="ob", bufs=4) as p_ob,
    ):
        wo_t = p_wo.tile([128, QCH, E], BF16, tag="wo")
        nc.gpsimd.dma_start(wo_t[:], d["wo"].ap())

        def oproj_qc(qq, gatc, q4):
            # One 128-token block of o_proj; et pairs so each gat stationary
            # is loaded once per pair while PSUM footprint stays at 2 banks.
            qc = 4 * qq + q4
            for eh in range(2):
                ops = []
                for e2 in range(2):
                    op = p_t1.tile([128, 512], F32, tag="t1")
                    ops.append(op)
                for hc in range(QCH):
                    for e2 in range(2):
                        nc.tensor.matmul(
                            ops[e2][:],
                            gatc[:, hc, ts(q4, 128)],
                            wo_t[:, hc, ts(2 * eh + e2, 512)],
                            start=(hc == 0),
                            stop=(hc == QCH - 1),
                        )
                for e2 in range(2):
                    et = 2 * eh + e2
                    ob = p_ob.tile([128, 512], F32, tag="ob")
                    nc.scalar.copy(ob[:], ops[e2][:])
                    eng = nc.sync if et % 2 == 0 else nc.scalar
                    eng.dma_start(d["out"].ap()[qc][:, ts(et, 512)], ob[:])

        def attn_head(qq, h, gatc):
            nk = 4 * qq + 4
            av2 = p_av.tile([128, 2, 512], F32, tag="av2")
            acc_a = p_acc.tile([128, 512], F32R, tag="acc")
            # Column 0's odd chunks are all partial-width (diagonal), so a
            # full-width copy-init of a second partial chain would read
            # stale data there; use a single chain for that short column.
            split = qq > 0
            if split:
                acc_b = p_acc.tile([128, 512], F32R, tag="acc")
            else:
                acc_b = None
            na = nb = 0
            for kk in range(nk):
                dg = kk - 4 * qq
                lo = 128 * dg if dg > 0 else 0
                w = 512 - lo
                sp = p_t1.tile([128, 512], F32, tag="t1")
                nc.tensor.matmul(
                    sp[:, ds(lo, w)],
                    kt[:, 0, ts(kk, 128)],
                    qT[:, 2 * h, ds(qq * 512 + lo, w)],
                    start=True, stop=False,
                )
                nc.tensor.matmul(
                    sp[:, ds(lo, w)],
                    kt[:, 1, ts(kk, 128)],
                    qT[:, 2 * h + 1, ds(qq * 512 + lo, w)],
                    start=False, stop=True,
                )
                ex = p_ex.tile([128, 512], BF16, tag="ex")
                nc.scalar.activation(
                    ex[:, ds(lo, w)], sp[:, ds(lo, w)], AF.Exp, scale=0.0625
                )
                if dg >= 0:
                    nc.vector.tensor_mul(
                        ex[:, ds(lo, 128)], ex[:, ds(lo, 128)], tri[:]
                    )
                # Two interleaved softmax-denominator partials: even chunks
                # accumulate on DVE, odd chunks on GpSimd — halves the serial
                # chain and keeps it off the PE-feeding DVE queue.
                # Denominator partials accumulate as two chains on two
                # different engines (DVE + GpSimd): each chain's serial
                # latency fits inside the column's PE time, and only half
                # the adds sit in the DVE FIFO ahead of the tri mask.
                if kk % 2 == 0 or not split:
                    if na == 0:
                        nc.vector.tensor_copy(acc_a[:], ex[:])
                    else:
                        nc.vector.tensor_add(
                            acc_a[:, ds(lo, w)], acc_a[:, ds(lo, w)],
                            ex[:, ds(lo, w)],
                        )
                    na += 1
                else:
                    if nb == 0:
                        nc.gpsimd.tensor_copy(acc_b[:], ex[:])
                    else:
                        nc.gpsimd.tensor_add(
                            acc_b[:, ds(lo, w)], acc_b[:, ds(lo, w)],
                            ex[:, ds(lo, w)],
                        )
                    nb += 1
                st, en = (kk == 0), (kk == nk - 1)
                nc.tensor.matmul(
                    av2[:, 0, ds(lo, w)], vt[:, kk, 0:128], ex[:, ds(lo, w)],
                    start=st, stop=en,
                )
                nc.tensor.matmul(
                    av2[:, 1, ds(lo, w)], vt[:, kk, 128:256], ex[:, ds(lo, w)],
                    start=st, stop=en,
                )
            sm = p_t1.tile([128, 512], F32, tag="t1")
            nc.tensor.matmul(sm[:], ones[:], acc_a[:], start=True, stop=not split)
            if split:
                nc.tensor.matmul(sm[:], ones[:], acc_b[:], start=False, stop=True)
            rec = p_vec.tile([128, 1, 512], F32, tag="rec")
            nc.vector.reciprocal(rec[:, 0, :], sm[:])
            g1 = p_vec.tile([128, 2, 512], F32, tag="g1")
            nc.vector.tensor_mul(
                g1[:], av2[:], gT[:, ds(2 * h, 2), ts(qq, 512)]
            )
            g1b, recb = bass.broadcast_tensor_aps(g1[:], rec[:])
            nc.vector.tensor_mul(gatc[:, ds(2 * h, 2), :], g1b, recb)

        prev = None
        for qq in (3, 2, 1, 0):
            gatc = p_gat.tile([128, QCH, 512], BF16, tag="gat")
            for h in range(NHC):
                attn_head(qq, h, gatc)
                # o_proj of the previous column interleaves one 128-token
                # block per head: independent PE work that fills the
                # exp/mask dependency bubbles of the attention chain.
                if prev is not None:
                    oproj_qc(prev[0], prev[1], h)
            prev = (qq, gatc)
        for q4 in range(4):
            oproj_qc(prev[0], prev[1], q4)

    stack.close()


def build_nc():
    nc = bacc.Bacc("TRN2", target_bir_lowering=False, debug=False)
    d = {}
    d["xt"] = nc.dram_tensor("xt", [128, ECH, S], BF16, kind="ExternalInput")
    d["wq"] = nc.dram_tensor("wq", [QCH, 128, ECH, 128], BF16, kind="ExternalInput")
    d["wg"] = nc.dram_tensor("wg", [QCH, 128, ECH, 128], BF16, kind="ExternalInput")
    d["wk"] = nc.dram_tensor("wk", [2, 128, ECH, 128], BF16, kind="ExternalInput")
    d["wv"] = nc.dram_tensor("wv", [128, ECH, D], BF16, kind="ExternalInput")
    d["wo"] = nc.dram_tensor("wo", [128, QCH, E], BF16, kind="ExternalInput")
    d["cost"] = nc.dram_tensor("cost", [RD, S], F32, kind="ExternalInput")
    d["sint"] = nc.dram_tensor("sint", [RD, S], F32, kind="ExternalInput")
    d["tri"] = nc.dram_tensor("tri", [128, 128], BF16, kind="ExternalInput")
    d["ones"] = nc.dram_tensor("ones", [128, 128], F32R, kind="ExternalInput")
    d["out"] = nc.dram_tensor("out", [NQC, 128, E], F32, kind="ExternalOutput")
    with tile.TileContext(nc) as tc:
        _body(tc, d)
    nc.compile()
    return nc


_NC_CACHE = None


def _get_nc():
    global _NC_CACHE
    if _NC_CACHE is None:
        _NC_CACHE = build_nc()
    return _NC_CACHE


def _rope_tables():
    inv = 1.0 / (10000.0 ** (np.arange(0, RD, 2, dtype=np.float32) / np.float32(RD)))
    t = np.arange(S, dtype=np.float32)
    freqs = np.outer(t, inv).astype(np.float32)          # [S, RD/2]
    emb = np.concatenate([freqs, freqs], axis=1)         # [S, RD]
    sinT = np.ascontiguousarray(np.sin(emb).astype(np.float32).T)
    # The rope rotation's sign lives in the sin table: the kernel builds
    # rot = [x_hi, x_lo] by partition swap and the -1 on the first half
    # of the rotated vector is folded in here.
    sinT[: RD // 2, :] *= -1.0
    return np.ascontiguousarray(np.cos(emb).astype(np.float32).T), sinT


def _tri():
    p = np.arange(128)[:, None]
    j = np.arange(128)[None, :]
    return (p <= j).astype(ml_dtypes.bfloat16)


def _prep_in_maps(hidden_states, Wq, Wk, Wv, Wg, Wo):
    cosT, sinT = _rope_tables()
    tri = _tri()
    maps = []
    for c in range(8):
        b, t = c // 4, c % 4
        hq0, kvh = 4 * t, (t // 2)
        cols = slice(hq0 * D, (hq0 + NHC) * D)
        kcols = slice(kvh * D, (kvh + 1) * D)
        x = hidden_states[b]  # [S, E]
        m = {
            "xt": np.ascontiguousarray(
                x.T.reshape(ECH, 128, S).transpose(1, 0, 2)
            ).astype(ml_dtypes.bfloat16),
            "wq": np.ascontiguousarray(
                Wq[:, cols].reshape(ECH, 128, QCH, 128).transpose(2, 1, 0, 3)
            ).astype(ml_dtypes.bfloat16),
            "wg": np.ascontiguousarray(
                Wg[:, cols].reshape(ECH, 128, QCH, 128).transpose(2, 1, 0, 3)
            ).astype(ml_dtypes.bfloat16),
            "wk": np.ascontiguousarray(
                Wk[:, kcols].reshape(ECH, 128, 2, 128).transpose(2, 1, 0, 3)
            ).astype(ml_dtypes.bfloat16),
            "wv": np.ascontiguousarray(
                Wv[:, kcols].reshape(ECH, 128, D).transpose(1, 0, 2)
            ).astype(ml_dtypes.bfloat16),
            "wo": np.ascontiguousarray(
                Wo[cols, :].reshape(QCH, 128, E).transpose(1, 0, 2)
            ).astype(ml_dtypes.bfloat16),
            "cost": cosT,
            "sint": sinT,
            "tri": tri,
            "ones": np.ones((128, 128), dtype=np.float32),
        }
        maps.append(m)
    return maps


def _run(inputs, trace=False, trace_cores=None, tmpdir=None):
    nc = _get_nc()
    in_maps = _prep_in_maps(**inputs)
    kw = {}
    if trace:
        kw = dict(trace=True, trace_cores=trace_cores, tmpdir=tmpdir)
    res = run_bass_kernel_spmd(nc, in_maps, list(range(8)), **kw)
    outs = [res.results[c]["out"].reshape(S, E) for c in range(8)]
    full = np.stack(
        [
            outs[0] + outs[1] + outs[2] + outs[3],
            outs[4] + outs[5] + outs[6] + outs[7],
        ]
    ).astype(np.float32)
    return full, res


def kernel(hidden_states, Wq, Wk, Wv, Wg, Wo):
    full, _ = _run(
        dict(hidden_states=np.asarray(hidden_states, dtype=np.float32),
             Wq=np.asarray(Wq, dtype=np.float32),
             Wk=np.asarray(Wk, dtype=np.float32),
             Wv=np.asarray(Wv, dtype=np.float32),
             Wg=np.asarray(Wg, dtype=np.float32),
             Wo=np.asarray(Wo, dtype=np.float32))
    )
    return full


if __name__ == "__main__":
    build_nc()
    print("build OK")


# revision 33
# speedup vs baseline: 1.2166x; 1.0316x over previous
"""Gated GQA attention block (B=2,S=2048,E=2048,H=16,HKV=2,D=256,RD=64) on 8 TRN2 cores.

Sharding: data-parallel on batch (2 groups of 4 cores); within a group,
tensor-parallel on query heads (4 heads/core). Each core computes its KV head's
k/v projection locally (duplicated across the 2 cores sharing a KV head).
o_proj is row-parallel; the all-reduce over the 4 cores of a group happens on
the host after gather.

All intermediates (q, g, k, v, gated attention output) stay SBUF-resident in
bf16 — no DRAM round trips between projections, attention, and o_proj.  bf16
matmuls run 1 cycle/row at any moving width, which lets the causal diagonal
restrict score/av matmuls to the live query range.  The softmax denominator is
accumulated chunk-wise on the vector engine and reduced with a single
ones-matmul per (head, column) instead of one per key chunk.
"""

import sys

if "/opt/trn_rl_repo" not in sys.path:
    sys.path.insert(0, "/opt/trn_rl_repo")

import ml_dtypes
import numpy as np

import concourse.bass as bass
import concourse.tile as tile
from concourse import bacc, mybir
from concourse.bass_utils import run_bass_kernel_spmd

F32 = mybir.dt.float32
F32R = mybir.dt.float32r
BF16 = mybir.dt.bfloat16
AF = mybir.ActivationFunctionType

S = 2048          # tokens per batch element
E = 2048          # model dim
D = 256           # head dim
RD = 64           # rope dims
NHC = 4           # q heads per core
HD = NHC * D      # per-core head dims (1024)
ECH = E // 128    # 16 contraction chunks
QCH = HD // 128   # 8 per-core q/g/o d-chunks
TT = 4            # 512-wide token tiles
NKC = S // 128    # 16 k chunks
NQC = S // 128    # 16 q chunks (oproj)


def _r(ap):
    return ap if ap.dtype in (F32R, BF16) else ap.bitcast(F32R)


RG = [[0, 1], [2, 3], [4, 5], [6, 7]]  # core pairs sharing a KV head


def _body(tc, d):
    nc = tc.nc
    ts = bass.ts
    ds = bass.ds

    from contextlib import ExitStack

    stack = ExitStack()

    p_res = stack.enter_context(tc.tile_pool(name="res", bufs=1))
    kt = p_res.tile([128, 2, S], BF16, tag="kt")
    vt = p_res.tile([128, NKC, D], BF16, tag="vt")
    qT = p_res.tile([128, QCH, S], BF16, tag="qT")
    gT = p_res.tile([128, QCH, S], BF16, tag="gT")

    p_c = stack.enter_context(tc.tile_pool(name="const", bufs=1))
    ones = p_c.tile([128, 128], F32R, tag="ones")
    nc.scalar.dma_start(ones[:], d["ones"].ap())
    tri = p_c.tile([128, 128], BF16, tag="tri")
    nc.scalar.dma_start(tri[:], d["tri"].ap())
    cos_t = p_c.tile([RD, S], F32, tag="cos")
    nc.scalar.dma_start(cos_t[:], d["cost"].ap())
    sin_t = p_c.tile([RD, S], F32, tag="sin")
    nc.scalar.dma_start(sin_t[:], d["sint"].ap())

    # k/v are needed by both cores of a pair; each core computes only its
    # d-half (host slices Wk/Wv per core) and an AllGather completes them.
    p_cc = stack.enter_context(tc.tile_pool(name="cc", bufs=1, space="DRAM"))
    kin = p_cc.tile([128, S], BF16, tag="kin")
    kout = p_cc.tile([2, 128, S], BF16, tag="kout")
    vin = p_cc.tile([128, NKC, 128], BF16, tag="vin")
    vout = p_cc.tile([2, 128, NKC, 128], BF16, tag="vout")

    # ---------------- Phase 1: projections ----------------
    with (
        tc.tile_pool(name="psum1", bufs=8, space="PSUM") as psum,
        tc.tile_pool(name="xt", bufs=1) as p_xt,
        tc.tile_pool(name="w", bufs=2) as p_w,
        tc.tile_pool(name="wv", bufs=1) as p_wv,
        tc.tile_pool(name="rtmp", bufs=2) as p_rtmp,
    ):
        wv_t = p_wv.tile([128, ECH, 128], BF16, tag="wv")
        for wh in range(4):
            nc.scalar.dma_start(
                wv_t[:, ts(wh, ECH // 4), :], d["wv"].ap()[:, ts(wh, ECH // 4), :]
            )
        vhalf = p_wv.tile([128, NKC, 128], BF16, tag="vhalf")

        # x load split across the sync HWDGE ring and the SWDGE ring;
        # the scalar HWDGE ring is reserved for the (latency-critical)
        # projection weights.
        xt = p_xt.tile([128, ECH, S], BF16, tag="xt")
        for ec in range(ECH):
            eng = nc.sync if ec % 2 == 0 else nc.gpsimd
            eng.dma_start(xt[:, ec, :], d["xt"].ap()[:, ec, :])

        def rope(sl, t):
            # sl[0:64, :512] holds rope dims (partition = d), bf16 in SBUF.
            # The half-rotation is a partition swap done by SBUF->SBUF DMA
            # (sign is folded into the sin table), keeping rope off the PE.
            half = RD // 2
            rot = p_rtmp.tile([RD, 512], BF16, tag="rot")
            nc.sync.dma_start(rot[0:half, :], sl[half:RD, :])
            nc.sync.dma_start(rot[half:RD, :], sl[0:half, :])
            tmp = p_rtmp.tile([RD, 512], F32, tag="rt")
            nc.vector.tensor_mul(tmp[:], sl[0:RD, :], cos_t[:, ts(t, 512)])
            nc.vector.tensor_mul(sl[0:RD, :], rot[:], sin_t[:, ts(t, 512)])
            nc.vector.tensor_add(sl[0:RD, :], sl[0:RD, :], tmp[:])

        def proj_chunk(w_ap, dst, idx, kind):
            wt = p_w.tile([128, ECH, 128], BF16, tag="w")
            nc.scalar.dma_start(wt[:], w_ap)
            pss = []
            for t in range(TT):
                pt = psum.tile([128, 512], F32, tag="ps")
                pss.append(pt)
            for ec in range(ECH):
                for t in range(TT):
                    nc.tensor.matmul(
                        pss[t][:],
                        wt[:, ec, :],
                        xt[:, ec, ts(t, 512)],
                        start=(ec == 0),
                        stop=(ec == ECH - 1),
                    )
            for t in range(TT):
                if kind == "kraw":
                    ktmp = p_w.tile([128, 512], BF16, tag="ktmp")
                    nc.scalar.copy(ktmp[:], pss[t][:])
                    nc.sync.dma_start(kin[:, ts(t, 512)], ktmp[:])
                    continue
                sl = dst[:, idx, ts(t, 512)]
                if kind == "g":
                    nc.scalar.activation(sl, pss[t][:], AF.Sigmoid)
                else:
                    nc.scalar.copy(sl, pss[t][:])
                    if kind == "q" and idx % 2 == 0:
                        rope(sl, t)

        # v first, ec-outer in two 8-bank PSUM waves: its matmuls consume each
        # xt chunk as it streams in, filling the DMA ramp.
        for wave in range(2):
            pss = []
            for i in range(8):
                pv = psum.tile([128, 128], F32, tag="ps")
                pss.append(pv)
            for ec in range(ECH):
                for i in range(8):
                    tcn = wave * 8 + i
                    nc.tensor.matmul(
                        pss[i][:],
                        xt[:, ec, ts(tcn, 128)],
                        wv_t[:, ec, :],
                        start=(ec == 0),
                        stop=(ec == ECH - 1),
                    )
            for i in range(8):
                tcn = wave * 8 + i
                nc.scalar.copy(vhalf[:, tcn, :], pss[i][:])
            nc.sync.dma_start(
                vin[:, ts(wave, 8), :], vhalf[:, ts(wave, 8), :]
            )
        nc.gpsimd.collective_compute(
            "AllGather", mybir.AluOpType.bypass, replica_groups=RG,
            ins=[vin[:].opt()], outs=[vout[:].opt()],
        )
        for j in range(2):
            nc.sync.dma_start(vt[:, :, ts(j, 128)], vout[j])

        proj_chunk(d["wk"].ap()[0], None, 0, "kraw")
        nc.gpsimd.collective_compute(
            "AllGather", mybir.AluOpType.bypass, replica_groups=RG,
            ins=[kin[:].opt()], outs=[kout[:].opt()],
        )
        for j in range(2):
            nc.sync.dma_start(kt[:, j, :], kout[j])
        for t in range(TT):
            rope(kt[:, 0, ts(t, 512)], t)

        for h in range(NHC):
            for jj in (2 * h, 2 * h + 1):
                proj_chunk(d["wq"].ap()[jj], qT, jj, "q")
            for jj in (2 * h, 2 * h + 1):
                proj_chunk(d["wg"].ap()[jj], gT, jj, "g")

    # ---------------- Phase 2: attention + o_proj ----------------
    with (
        tc.tile_pool(name="pav", bufs=2, space="PSUM") as p_av,
        tc.tile_pool(name="pt1", bufs=4, space="PSUM") as p_t1,
        tc.tile_pool(name="wo", bufs=1) as p_wo,
        tc.tile_pool(name="gat", bufs=2) as p_gat,
        tc.tile_pool(name="ex", bufs=10) as p_ex,
        tc.tile_pool(name="acc", bufs=4) as p_acc,
        tc.tile_pool(name="vec", bufs=4) as p_vec,
        tc.tile_pool(name="ob", bufs=4) as p_ob,
    ):
        wo_t = p_wo.tile([128, QCH, E], BF16, tag="wo")
        nc.gpsimd.dma_start(wo_t[:], d["wo"].ap())

        def oproj_qc(qq, gatc, q4):
            # One 128-token block of o_proj; et pairs so each gat stationary
            # is loaded once per pair while PSUM footprint stays at 2 banks.
            qc = 4 * qq + q4
            for eh in range(2):
                ops = []
                for e2 in range(2):
                    op = p_t1.tile([128, 512], F32, tag="t1")
                    ops.append(op)
                for hc in range(QCH):
                    for e2 in range(2):
                        nc.tensor.matmul(
                            ops[e2][:],
                            gatc[:, hc, ts(q4, 128)],
                            wo_t[:, hc, ts(2 * eh + e2, 512)],
                            start=(hc == 0),
                            stop=(hc == QCH - 1),
                        )
                for e2 in range(2):
                    et = 2 * eh + e2
                    ob = p_ob.tile([128, 512], F32, tag="ob")
                    nc.scalar.copy(ob[:], ops[e2][:])
                    eng = nc.sync if et % 2 == 0 else nc.scalar
                    eng.dma_start(d["out"].ap()[qc][:, ts(et, 512)], ob[:])

        def attn_head(qq, h, gatc):
            nk = 4 * qq + 4
            av2 = p_av.tile([128, 2, 512], F32, tag="av2")
            acc_a = p_acc.tile([128, 512], F32R, tag="acc")
            # Column 0's odd chunks are all partial-width (diagonal), so a
            # full-width copy-init of a second partial chain would read
            # stale data there; use a single chain for that short column.
            split = qq > 0
            if split:
                acc_b = p_acc.tile([128, 512], F32R, tag="acc")
            else:
                acc_b = None
            na = nb = 0
            for kk in range(nk):
                dg = kk - 4 * qq
                lo = 128 * dg if dg > 0 else 0
                w = 512 - lo
                sp = p_t1.tile([128, 512], F32, tag="t1")
                nc.tensor.matmul(
                    sp[:, ds(lo, w)],
                    kt[:, 0, ts(kk, 128)],
                    qT[:, 2 * h, ds(qq * 512 + lo, w)],
                    start=True, stop=False,
                )
                nc.tensor.matmul(
                    sp[:, ds(lo, w)],
                    kt[:, 1, ts(kk, 128)],
                    qT[:, 2 * h + 1, ds(qq * 512 + lo, w)],
                    start=False, stop=True,
                )
                ex = p_ex.tile([128, 512], BF16, tag="ex")
                nc.scalar.activation(
                    ex[:, ds(lo, w)], sp[:, ds(lo, w)], AF.Exp, scale=0.0625
                )
                if dg >= 0:
                    nc.vector.tensor_mul(
                        ex[:, ds(lo, 128)], ex[:, ds(lo, 128)], tri[:]
                    )
                # Two interleaved softmax-denominator partials: even chunks
                # accumulate on DVE, odd chunks on GpSimd — halves the serial
                # chain and keeps it off the PE-feeding DVE queue.
                # Denominator partials accumulate as two chains on two
                # different engines (DVE + GpSimd): each chain's serial
                # latency fits inside the column's PE time, and only half
                # the adds sit in the DVE FIFO ahead of the tri mask.
                if kk % 2 == 0 or not split:
                    if na == 0:
                        nc.vector.tensor_copy(acc_a[:], ex[:])
                    else:
                        nc.vector.tensor_add(
                            acc_a[:, ds(lo, w)], acc_a[:, ds(lo, w)],
                            ex[:, ds(lo, w)],
                        )
                    na += 1
                else:
                    if nb == 0:
                        nc.gpsimd.tensor_copy(acc_b[:], ex[:])
                    else:
                        nc.gpsimd.tensor_add(
                            acc_b[:, ds(lo, w)], acc_b[:, ds(lo, w)],
                            ex[:, ds(lo, w)],
                        )
                    nb += 1
                st, en = (kk == 0), (kk == nk - 1)
                nc.tensor.matmul(
                    av2[:, 0, ds(lo, w)], vt[:, kk, 0:128], ex[:, ds(lo, w)],
                    start=st, stop=en,
                )
                nc.tensor.matmul(
                    av2[:, 1, ds(lo, w)], vt[:, kk, 128:256], ex[:, ds(lo, w)],
                    start=st, stop=en,
                )
            sm = p_t1.tile([128, 512], F32, tag="t1")
            nc.tensor.matmul(sm[:], ones[:], acc_a[:], start=True, stop=not split)
            if split:
                nc.tensor.matmul(sm[:], ones[:], acc_b[:], start=False, stop=True)
            rec = p_vec.tile([128, 1, 512], F32, tag="rec")
            nc.vector.reciprocal(rec[:, 0, :], sm[:])
            g1 = p_vec.tile([128, 2, 512], F32, tag="g1")
            nc.vector.tensor_mul(
                g1[:], av2[:], gT[:, ds(2 * h, 2), ts(qq, 512)]
            )
            g1b, recb = bass.broadcast_tensor_aps(g1[:], rec[:])
            nc.vector.tensor_mul(gatc[:, ds(2 * h, 2), :], g1b, recb)

        prev = None
        for qq in (3, 2, 1, 0):
            gatc = p_gat.tile([128, QCH, 512], BF16, tag="gat")
            for h in range(NHC):
                attn_head(qq, h, gatc)
                # o_proj of the previous column interleaves one 128-token
                # block per head: independent PE work that fills the
                # exp/mask dependency bubbles of the attention chain.
                if prev is not None:
                    oproj_qc(prev[0], prev[1], h)
            prev = (qq, gatc)
        for q4 in range(4):
            oproj_qc(prev[0], prev[1], q4)

    stack.close()


def build_nc():
    nc = bacc.Bacc("TRN2", target_bir_lowering=False, debug=False, num_devices=8)
    d = {}
    d["xt"] = nc.dram_tensor("xt", [128, ECH, S], BF16, kind="ExternalInput")
    d["wq"] = nc.dram_tensor("wq", [QCH, 128, ECH, 128], BF16, kind="ExternalInput")
    d["wg"] = nc.dram_tensor("wg", [QCH, 128, ECH, 128], BF16, kind="ExternalInput")
    d["wk"] = nc.dram_tensor("wk", [1, 128, ECH, 128], BF16, kind="ExternalInput")
    d["wv"] = nc.dram_tensor("wv", [128, ECH, 128], BF16, kind="ExternalInput")
    d["wo"] = nc.dram_tensor("wo", [128, QCH, E], BF16, kind="ExternalInput")
    d["cost"] = nc.dram_tensor("cost", [RD, S], F32, kind="ExternalInput")
    d["sint"] = nc.dram_tensor("sint", [RD, S], F32, kind="ExternalInput")
    d["tri"] = nc.dram_tensor("tri", [128, 128], BF16, kind="ExternalInput")
    d["ones"] = nc.dram_tensor("ones", [128, 128], F32R, kind="ExternalInput")
    d["out"] = nc.dram_tensor("out", [NQC, 128, E], F32, kind="ExternalOutput")
    with tile.TileContext(nc) as tc:
        _body(tc, d)
    nc.compile()
    return nc


_NC_CACHE = None


def _get_nc():
    global _NC_CACHE
    if _NC_CACHE is None:
        _NC_CACHE = build_nc()
    return _NC_CACHE


def _rope_tables():
    inv = 1.0 / (10000.0 ** (np.arange(0, RD, 2, dtype=np.float32) / np.float32(RD)))
    t = np.arange(S, dtype=np.float32)
    freqs = np.outer(t, inv).astype(np.float32)          # [S, RD/2]
    emb = np.concatenate([freqs, freqs], axis=1)         # [S, RD]
    sinT = np.ascontiguousarray(np.sin(emb).astype(np.float32).T)
    # The rope rotation's sign lives in the sin table: the kernel builds
    # rot = [x_hi, x_lo] by partition swap and the -1 on the first half
    # of the rotated vector is folded in here.
    sinT[: RD // 2, :] *= -1.0
    return np.ascontiguousarray(np.cos(emb).astype(np.float32).T), sinT


def _tri():
    p = np.arange(128)[:, None]
    j = np.arange(128)[None, :]
    return (p <= j).astype(ml_dtypes.bfloat16)


def _prep_in_maps(hidden_states, Wq, Wk, Wv, Wg, Wo):
    cosT, sinT = _rope_tables()
    tri = _tri()
    maps = []
    for c in range(8):
        b, t = c // 4, c % 4
        hq0, kvh = 4 * t, (t // 2)
        cols = slice(hq0 * D, (hq0 + NHC) * D)
        # this core's d-half of its KV head's k/v projection (pair rank t%2)
        k0 = kvh * D + (t % 2) * 128
        khalf = slice(k0, k0 + 128)
        x = hidden_states[b]  # [S, E]
        m = {
            "xt": np.ascontiguousarray(
                x.T.reshape(ECH, 128, S).transpose(1, 0, 2)
            ).astype(ml_dtypes.bfloat16),
            "wq": np.ascontiguousarray(
                Wq[:, cols].reshape(ECH, 128, QCH, 128).transpose(2, 1, 0, 3)
            ).astype(ml_dtypes.bfloat16),
            "wg": np.ascontiguousarray(
                Wg[:, cols].reshape(ECH, 128, QCH, 128).transpose(2, 1, 0, 3)
            ).astype(ml_dtypes.bfloat16),
            "wk": np.ascontiguousarray(
                Wk[:, khalf].reshape(ECH, 128, 1, 128).transpose(2, 1, 0, 3)
            ).astype(ml_dtypes.bfloat16),
            "wv": np.ascontiguousarray(
                Wv[:, khalf].reshape(ECH, 128, 128).transpose(1, 0, 2)
            ).astype(ml_dtypes.bfloat16),
            "wo": np.ascontiguousarray(
                Wo[cols, :].reshape(QCH, 128, E).transpose(1, 0, 2)
            ).astype(ml_dtypes.bfloat16),
            "cost": cosT,
            "sint": sinT,
            "tri": tri,
            "ones": np.ones((128, 128), dtype=np.float32),
        }
        maps.append(m)
    return maps


def _run(inputs, trace=False, trace_cores=None, tmpdir=None):
    nc = _get_nc()
    in_maps = _prep_in_maps(**inputs)
    kw = {}
    if trace:
        kw = dict(trace=True, trace_cores=trace_cores, tmpdir=tmpdir)
    res = run_bass_kernel_spmd(nc, in_maps, list(range(8)), **kw)
    outs = [res.results[c]["out"].reshape(S, E) for c in range(8)]
    full = np.stack(
        [
            outs[0] + outs[1] + outs[2] + outs[3],
            outs[4] + outs[5] + outs[6] + outs[7],
        ]
    ).astype(np.float32)
    return full, res


def kernel(hidden_states, Wq, Wk, Wv, Wg, Wo):
    full, _ = _run(
        dict(hidden_states=np.asarray(hidden_states, dtype=np.float32),
             Wq=np.asarray(Wq, dtype=np.float32),
             Wk=np.asarray(Wk, dtype=np.float32),
             Wv=np.asarray(Wv, dtype=np.float32),
             Wg=np.asarray(Wg, dtype=np.float32),
             Wo=np.asarray(Wo, dtype=np.float32))
    )
    return full


if __name__ == "__main__":
    build_nc()
    print("build OK")


# revision 38
# speedup vs baseline: 1.2247x; 1.0067x over previous
"""Gated GQA attention block (B=2,S=2048,E=2048,H=16,HKV=2,D=256,RD=64) on 8 TRN2 cores.

Sharding: data-parallel on batch (2 groups of 4 cores); within a group,
tensor-parallel on query heads (4 heads/core). Each core computes its KV head's
k/v projection locally (duplicated across the 2 cores sharing a KV head).
o_proj is row-parallel; the all-reduce over the 4 cores of a group happens on
the host after gather.

All intermediates (q, g, k, v, gated attention output) stay SBUF-resident in
bf16 — no DRAM round trips between projections, attention, and o_proj.  bf16
matmuls run 1 cycle/row at any moving width, which lets the causal diagonal
restrict score/av matmuls to the live query range.  The softmax denominator is
accumulated chunk-wise on the vector engine and reduced with a single
ones-matmul per (head, column) instead of one per key chunk.
"""

import sys

if "/opt/trn_rl_repo" not in sys.path:
    sys.path.insert(0, "/opt/trn_rl_repo")

import ml_dtypes
import numpy as np

import concourse.bass as bass
import concourse.tile as tile
from concourse import bacc, mybir
from concourse.bass_utils import run_bass_kernel_spmd

F32 = mybir.dt.float32
F32R = mybir.dt.float32r
BF16 = mybir.dt.bfloat16
AF = mybir.ActivationFunctionType

S = 2048          # tokens per batch element
E = 2048          # model dim
D = 256           # head dim
RD = 64           # rope dims
NHC = 4           # q heads per core
HD = NHC * D      # per-core head dims (1024)
ECH = E // 128    # 16 contraction chunks
QCH = HD // 128   # 8 per-core q/g/o d-chunks
TT = 4            # 512-wide token tiles
NKC = S // 128    # 16 k chunks
NQC = S // 128    # 16 q chunks (oproj)


def _r(ap):
    return ap if ap.dtype in (F32R, BF16) else ap.bitcast(F32R)


RG = [[0, 1], [2, 3], [4, 5], [6, 7]]  # core pairs sharing a KV head


def _body(tc, d):
    nc = tc.nc
    ts = bass.ts
    ds = bass.ds

    from contextlib import ExitStack

    stack = ExitStack()

    p_res = stack.enter_context(tc.tile_pool(name="res", bufs=1))
    kt = p_res.tile([128, 2, S], BF16, tag="kt")
    vt = p_res.tile([128, NKC, D], BF16, tag="vt")
    qT = p_res.tile([128, QCH, S], BF16, tag="qT")
    gT = p_res.tile([128, QCH, S], BF16, tag="gT")

    p_c = stack.enter_context(tc.tile_pool(name="const", bufs=1))
    ones = p_c.tile([128, 128], F32R, tag="ones")
    nc.scalar.dma_start(ones[:], d["ones"].ap())
    tri = p_c.tile([128, 128], BF16, tag="tri")
    nc.scalar.dma_start(tri[:], d["tri"].ap())
    cos_t = p_c.tile([RD, S], F32, tag="cos")
    nc.scalar.dma_start(cos_t[:], d["cost"].ap())
    sin_t = p_c.tile([RD, S], F32, tag="sin")
    nc.scalar.dma_start(sin_t[:], d["sint"].ap())

    # k/v are needed by both cores of a pair; each core computes only its
    # d-half (host slices Wk/Wv per core) and an AllGather completes them.
    p_cc = stack.enter_context(tc.tile_pool(name="cc", bufs=1, space="DRAM"))
    kin = p_cc.tile([128, S], BF16, tag="kin")
    kout = p_cc.tile([2, 128, S], BF16, tag="kout")
    vin = p_cc.tile([128, NKC, 128], BF16, tag="vin")
    vout = p_cc.tile([2, 128, NKC, 128], BF16, tag="vout")

    # ---------------- Phase 1: projections ----------------
    with (
        tc.tile_pool(name="psum1", bufs=8, space="PSUM") as psum,
        tc.tile_pool(name="xt", bufs=1) as p_xt,
        tc.tile_pool(name="w", bufs=4) as p_w,
        tc.tile_pool(name="wv", bufs=1) as p_wv,
        tc.tile_pool(name="rtmp", bufs=2) as p_rtmp,
    ):
        wv_t = p_wv.tile([128, ECH, 128], BF16, tag="wv")
        for wh in range(4):
            nc.scalar.dma_start(
                wv_t[:, ts(wh, ECH // 4), :], d["wv"].ap()[:, ts(wh, ECH // 4), :]
            )
        vhalf = p_wv.tile([128, NKC, 128], BF16, tag="vhalf")

        # x load split across both fast HWDGE rings (it paces the ramp);
        # the small wv/wk weights are queued on scalar ahead of the odd
        # x chunks, and the bulky wq/wg stream rides the SWDGE ring.
        xt = p_xt.tile([128, ECH, S], BF16, tag="xt")
        for ec in range(ECH):
            eng = nc.sync if ec % 2 == 0 else nc.scalar
            eng.dma_start(xt[:, ec, :], d["xt"].ap()[:, ec, :])

        def rope(sl, t):
            # sl[0:64, :512] holds rope dims (partition = d), bf16 in SBUF.
            # The half-rotation is a partition swap done by SBUF->SBUF DMA
            # (sign is folded into the sin table), keeping rope off the PE.
            half = RD // 2
            rot = p_rtmp.tile([RD, 512], BF16, tag="rot")
            nc.sync.dma_start(rot[0:half, :], sl[half:RD, :])
            nc.sync.dma_start(rot[half:RD, :], sl[0:half, :])
            tmp = p_rtmp.tile([RD, 512], F32, tag="rt")
            nc.vector.tensor_mul(tmp[:], sl[0:RD, :], cos_t[:, ts(t, 512)])
            nc.vector.tensor_mul(sl[0:RD, :], rot[:], sin_t[:, ts(t, 512)])
            nc.vector.tensor_add(sl[0:RD, :], sl[0:RD, :], tmp[:])

        def proj_chunk(w_ap, dst, idx, kind):
            wt = p_w.tile([128, ECH, 128], BF16, tag="w")
            weng = nc.scalar if kind == "kraw" else nc.gpsimd
            weng.dma_start(wt[:], w_ap)
            pss = []
            for t in range(TT):
                pt = psum.tile([128, 512], F32, tag="ps")
                pss.append(pt)
            for ec in range(ECH):
                for t in range(TT):
                    nc.tensor.matmul(
                        pss[t][:],
                        wt[:, ec, :],
                        xt[:, ec, ts(t, 512)],
                        start=(ec == 0),
                        stop=(ec == ECH - 1),
                    )
            for t in range(TT):
                if kind == "kraw":
                    ktmp = p_w.tile([128, 512], BF16, tag="ktmp")
                    nc.scalar.copy(ktmp[:], pss[t][:])
                    nc.sync.dma_start(kin[:, ts(t, 512)], ktmp[:])
                    continue
                sl = dst[:, idx, ts(t, 512)]
                if kind == "g":
                    nc.scalar.activation(sl, pss[t][:], AF.Sigmoid)
                else:
                    nc.scalar.copy(sl, pss[t][:])
                    if kind == "q" and idx % 2 == 0:
                        rope(sl, t)

        # v first, ec-outer in two 8-bank PSUM waves: its matmuls consume each
        # xt chunk as it streams in, filling the DMA ramp.
        for wave in range(2):
            pss = []
            for i in range(8):
                pv = psum.tile([128, 128], F32, tag="ps")
                pss.append(pv)
            for ec in range(ECH):
                for i in range(8):
                    tcn = wave * 8 + i
                    nc.tensor.matmul(
                        pss[i][:],
                        xt[:, ec, ts(tcn, 128)],
                        wv_t[:, ec, :],
                        start=(ec == 0),
                        stop=(ec == ECH - 1),
                    )
            for i in range(8):
                tcn = wave * 8 + i
                nc.scalar.copy(vhalf[:, tcn, :], pss[i][:])
            nc.sync.dma_start(
                vin[:, ts(wave, 8), :], vhalf[:, ts(wave, 8), :]
            )
        nc.gpsimd.collective_compute(
            "AllGather", mybir.AluOpType.bypass, replica_groups=RG,
            ins=[vin[:].opt()], outs=[vout[:].opt()],
        )
        for j in range(2):
            nc.sync.dma_start(vt[:, :, ts(j, 128)], vout[j])

        proj_chunk(d["wk"].ap()[0], None, 0, "kraw")
        nc.gpsimd.collective_compute(
            "AllGather", mybir.AluOpType.bypass, replica_groups=RG,
            ins=[kin[:].opt()], outs=[kout[:].opt()],
        )
        for j in range(2):
            nc.sync.dma_start(kt[:, j, :], kout[j])
        for t in range(TT):
            rope(kt[:, 0, ts(t, 512)], t)

        for h in range(NHC):
            for jj in (2 * h, 2 * h + 1):
                proj_chunk(d["wq"].ap()[jj], qT, jj, "q")
            for jj in (2 * h, 2 * h + 1):
                proj_chunk(d["wg"].ap()[jj], gT, jj, "g")

    # ---------------- Phase 2: attention + o_proj ----------------
    with (
        tc.tile_pool(name="pav", bufs=2, space="PSUM") as p_av,
        tc.tile_pool(name="pt1", bufs=4, space="PSUM") as p_t1,
        tc.tile_pool(name="wo", bufs=1) as p_wo,
        tc.tile_pool(name="gat", bufs=2) as p_gat,
        tc.tile_pool(name="ex", bufs=10) as p_ex,
        tc.tile_pool(name="acc", bufs=4) as p_acc,
        tc.tile_pool(name="vec", bufs=4) as p_vec,
        tc.tile_pool(name="ob", bufs=4) as p_ob,
    ):
        wo_t = p_wo.tile([128, QCH, E], BF16, tag="wo")
        nc.gpsimd.dma_start(wo_t[:], d["wo"].ap())

        def oproj_qc(qq, gatc, q4):
            # One 128-token block of o_proj; single PSUM slot at a time so
            # the shared t1 pool keeps slots free for attention lookahead.
            qc = 4 * qq + q4
            for et in range(4):
                op = p_t1.tile([128, 512], F32, tag="t1")
                for hc in range(QCH):
                    nc.tensor.matmul(
                        op[:],
                        gatc[:, hc, ts(q4, 128)],
                        wo_t[:, hc, ts(et, 512)],
                        start=(hc == 0),
                        stop=(hc == QCH - 1),
                    )
                ob = p_ob.tile([128, 512], F32, tag="ob")
                nc.scalar.copy(ob[:], op[:])
                eng = nc.sync if et % 2 == 0 else nc.scalar
                eng.dma_start(d["out"].ap()[qc][:, ts(et, 512)], ob[:])

        def attn_head(qq, h, gatc):
            nk = 4 * qq + 4
            av2 = p_av.tile([128, 2, 512], F32, tag="av2")
            acc_a = p_acc.tile([128, 512], F32R, tag="acc")
            # Column 0's odd chunks are all partial-width (diagonal), so a
            # full-width copy-init of a second partial chain would read
            # stale data there; use a single chain for that short column.
            split = qq > 0
            if split:
                acc_b = p_acc.tile([128, 512], F32R, tag="acc")
            else:
                acc_b = None
            na = nb = 0
            for kk in range(nk):
                dg = kk - 4 * qq
                lo = 128 * dg if dg > 0 else 0
                w = 512 - lo
                sp = p_t1.tile([128, 512], F32, tag="t1")
                nc.tensor.matmul(
                    sp[:, ds(lo, w)],
                    kt[:, 0, ts(kk, 128)],
                    qT[:, 2 * h, ds(qq * 512 + lo, w)],
                    start=True, stop=False,
                )
                nc.tensor.matmul(
                    sp[:, ds(lo, w)],
                    kt[:, 1, ts(kk, 128)],
                    qT[:, 2 * h + 1, ds(qq * 512 + lo, w)],
                    start=False, stop=True,
                )
                ex = p_ex.tile([128, 512], BF16, tag="ex")
                nc.scalar.activation(
                    ex[:, ds(lo, w)], sp[:, ds(lo, w)], AF.Exp, scale=0.0625
                )
                if dg >= 0:
                    nc.vector.tensor_mul(
                        ex[:, ds(lo, 128)], ex[:, ds(lo, 128)], tri[:]
                    )
                # Two interleaved softmax-denominator partials: even chunks
                # accumulate on DVE, odd chunks on GpSimd — halves the serial
                # chain and keeps it off the PE-feeding DVE queue.
                # Denominator partials accumulate as two chains on two
                # different engines (DVE + GpSimd): each chain's serial
                # latency fits inside the column's PE time, and only half
                # the adds sit in the DVE FIFO ahead of the tri mask.
                if kk % 2 == 0 or not split:
                    if na == 0:
                        nc.vector.tensor_copy(acc_a[:], ex[:])
                    else:
                        nc.vector.tensor_add(
                            acc_a[:, ds(lo, w)], acc_a[:, ds(lo, w)],
                            ex[:, ds(lo, w)],
                        )
                    na += 1
                else:
                    if nb == 0:
                        nc.gpsimd.tensor_copy(acc_b[:], ex[:])
                    else:
                        nc.gpsimd.tensor_add(
                            acc_b[:, ds(lo, w)], acc_b[:, ds(lo, w)],
                            ex[:, ds(lo, w)],
                        )
                    nb += 1
                st, en = (kk == 0), (kk == nk - 1)
                nc.tensor.matmul(
                    av2[:, 0, ds(lo, w)], vt[:, kk, 0:128], ex[:, ds(lo, w)],
                    start=st, stop=en,
                )
                nc.tensor.matmul(
                    av2[:, 1, ds(lo, w)], vt[:, kk, 128:256], ex[:, ds(lo, w)],
                    start=st, stop=en,
                )
            sm = p_t1.tile([128, 512], F32, tag="t1")
            nc.tensor.matmul(sm[:], ones[:], acc_a[:], start=True, stop=not split)
            if split:
                nc.tensor.matmul(sm[:], ones[:], acc_b[:], start=False, stop=True)
            rec = p_vec.tile([128, 1, 512], F32, tag="rec")
            nc.vector.reciprocal(rec[:, 0, :], sm[:])
            g1 = p_vec.tile([128, 2, 512], F32, tag="g1")
            nc.vector.tensor_mul(
                g1[:], av2[:], gT[:, ds(2 * h, 2), ts(qq, 512)]
            )
            g1b, recb = bass.broadcast_tensor_aps(g1[:], rec[:])
            nc.vector.tensor_mul(gatc[:, ds(2 * h, 2), :], g1b, recb)

        # o_proj of the previous column interleaves one 128-token block per
        # head, shifted one head late so the last block lands after the
        # current column's final head and fills its gating-chain bubble.
        prev = None
        for qq in (3, 2, 1, 0):
            gatc = p_gat.tile([128, QCH, 512], BF16, tag="gat")
            for h in range(NHC):
                attn_head(qq, h, gatc)
                if prev is not None and h > 0:
                    oproj_qc(prev[0], prev[1], h - 1)
            if prev is not None:
                oproj_qc(prev[0], prev[1], 3)
            prev = (qq, gatc)
        for q4 in range(4):
            oproj_qc(prev[0], prev[1], q4)

    stack.close()


def build_nc():
    nc = bacc.Bacc("TRN2", target_bir_lowering=False, debug=False, num_devices=8)
    d = {}
    d["xt"] = nc.dram_tensor("xt", [128, ECH, S], BF16, kind="ExternalInput")
    d["wq"] = nc.dram_tensor("wq", [QCH, 128, ECH, 128], BF16, kind="ExternalInput")
    d["wg"] = nc.dram_tensor("wg", [QCH, 128, ECH, 128], BF16, kind="ExternalInput")
    d["wk"] = nc.dram_tensor("wk", [1, 128, ECH, 128], BF16, kind="ExternalInput")
    d["wv"] = nc.dram_tensor("wv", [128, ECH, 128], BF16, kind="ExternalInput")
    d["wo"] = nc.dram_tensor("wo", [128, QCH, E], BF16, kind="ExternalInput")
    d["cost"] = nc.dram_tensor("cost", [RD, S], F32, kind="ExternalInput")
    d["sint"] = nc.dram_tensor("sint", [RD, S], F32, kind="ExternalInput")
    d["tri"] = nc.dram_tensor("tri", [128, 128], BF16, kind="ExternalInput")
    d["ones"] = nc.dram_tensor("ones", [128, 128], F32R, kind="ExternalInput")
    d["out"] = nc.dram_tensor("out", [NQC, 128, E], F32, kind="ExternalOutput")
    with tile.TileContext(nc) as tc:
        _body(tc, d)
    nc.compile()
    return nc


_NC_CACHE = None


def _get_nc():
    global _NC_CACHE
    if _NC_CACHE is None:
        _NC_CACHE = build_nc()
    return _NC_CACHE


def _rope_tables():
    inv = 1.0 / (10000.0 ** (np.arange(0, RD, 2, dtype=np.float32) / np.float32(RD)))
    t = np.arange(S, dtype=np.float32)
    freqs = np.outer(t, inv).astype(np.float32)          # [S, RD/2]
    emb = np.concatenate([freqs, freqs], axis=1)         # [S, RD]
    sinT = np.ascontiguousarray(np.sin(emb).astype(np.float32).T)
    # The rope rotation's sign lives in the sin table: the kernel builds
    # rot = [x_hi, x_lo] by partition swap and the -1 on the first half
    # of the rotated vector is folded in here.
    sinT[: RD // 2, :] *= -1.0
    return np.ascontiguousarray(np.cos(emb).astype(np.float32).T), sinT


def _tri():
    p = np.arange(128)[:, None]
    j = np.arange(128)[None, :]
    return (p <= j).astype(ml_dtypes.bfloat16)


def _prep_in_maps(hidden_states, Wq, Wk, Wv, Wg, Wo):
    cosT, sinT = _rope_tables()
    tri = _tri()
    maps = []
    for c in range(8):
        b, t = c // 4, c % 4
        hq0, kvh = 4 * t, (t // 2)
        cols = slice(hq0 * D, (hq0 + NHC) * D)
        # this core's d-half of its KV head's k/v projection (pair rank t%2)
        k0 = kvh * D + (t % 2) * 128
        khalf = slice(k0, k0 + 128)
        x = hidden_states[b]  # [S, E]
        m = {
            "xt": np.ascontiguousarray(
                x.T.reshape(ECH, 128, S).transpose(1, 0, 2)
            ).astype(ml_dtypes.bfloat16),
            "wq": np.ascontiguousarray(
                Wq[:, cols].reshape(ECH, 128, QCH, 128).transpose(2, 1, 0, 3)
            ).astype(ml_dtypes.bfloat16),
            "wg": np.ascontiguousarray(
                Wg[:, cols].reshape(ECH, 128, QCH, 128).transpose(2, 1, 0, 3)
            ).astype(ml_dtypes.bfloat16),
            "wk": np.ascontiguousarray(
                Wk[:, khalf].reshape(ECH, 128, 1, 128).transpose(2, 1, 0, 3)
            ).astype(ml_dtypes.bfloat16),
            "wv": np.ascontiguousarray(
                Wv[:, khalf].reshape(ECH, 128, 128).transpose(1, 0, 2)
            ).astype(ml_dtypes.bfloat16),
            "wo": np.ascontiguousarray(
                Wo[cols, :].reshape(QCH, 128, E).transpose(1, 0, 2)
            ).astype(ml_dtypes.bfloat16),
            "cost": cosT,
            "sint": sinT,
            "tri": tri,
            "ones": np.ones((128, 128), dtype=np.float32),
        }
        maps.append(m)
    return maps


def _run(inputs, trace=False, trace_cores=None, tmpdir=None):
    nc = _get_nc()
    in_maps = _prep_in_maps(**inputs)
    kw = {}
    if trace:
        kw = dict(trace=True, trace_cores=trace_cores, tmpdir=tmpdir)
    res = run_bass_kernel_spmd(nc, in_maps, list(range(8)), **kw)
    outs = [res.results[c]["out"].reshape(S, E) for c in range(8)]
    full = np.stack(
        [
            outs[0] + outs[1] + outs[2] + outs[3],
            outs[4] + outs[5] + outs[6] + outs[7],
        ]
    ).astype(np.float32)
    return full, res


def kernel(hidden_states, Wq, Wk, Wv, Wg, Wo):
    full, _ = _run(
        dict(hidden_states=np.asarray(hidden_states, dtype=np.float32),
             Wq=np.asarray(Wq, dtype=np.float32),
             Wk=np.asarray(Wk, dtype=np.float32),
             Wv=np.asarray(Wv, dtype=np.float32),
             Wg=np.asarray(Wg, dtype=np.float32),
             Wo=np.asarray(Wo, dtype=np.float32))
    )
    return full


if __name__ == "__main__":
    build_nc()
    print("build OK")
